# revision 26
# baseline (speedup 1.0000x reference)
"""Trainium2 Bass kernel for nn_LinearCategoricalEncoding (normalizing-flow stack).

Sharding: pure data parallel over batch B=64 -> 8 NeuronCores x 8 batches.
Per core: 16384 tokens, processed in 16 super-tiles of 1024 tokens
(two 512-token halves A/B so PE weight loads amortize and elementwise
ops run A/B-merged at [*, 1024] free size).

On-chip layout (per 512-token half, feature-major):
  z lives at partitions 64..127 of a [128, 1024] tile (chans 0-31 at
  p64..95, chans 32-63 at p96..127; A = cols 0..511, B = cols 512..1023).
  h0 (coupling MLP input) = [ext(p0..63); z chans0-31(p64..95); ones(p96)]
  so W0 @ h0 folds b0 via the ones row (K=97).

Matmuls use float32r (full fp32, 1 cycle/column at N=512). The scalar
engine only ever uses the gelu_and_others table set (Gelu + Tanh): exp is
computed exactly as exp(x) = (1+t)/(1-t), t = tanh(x/2), with the DVE
reciprocal_approx_fast (args bounded, rel err ~4e-6).

log-det-jacobian: per-(super-tile, flow) partial row sums come free from
activation accum_out into columns of an accumulator tile; one reduce +
ones-matmul folds them to per-batch scalars. The constant S*logdet(conv_W)
term is added on host.
"""

import os
import sys
from contextlib import ExitStack

import numpy as np

for _p in ("/opt/trn_rl_repo", "/root/.axon_site/_ro/trn_rl_repo"):
    if os.path.isdir(_p) and _p not in sys.path:
        sys.path.append(_p)

import concourse.bass as bass
import concourse.tile as tile
from concourse import mybir
from concourse.bass_utils import run_bass_kernel_spmd

B, S, D, E, H, F, V = 64, 2048, 64, 64, 512, 4, 32000
NCORE = 8
BPC = B // NCORE            # batches per core
TOK = BPC * S               # tokens per core
TT = 512                    # tokens per MLP tile (half of a super-tile)
SUP = 2 * TT
NSUP = TOK // SUP           # super-tiles per core
NG = TOK // 128             # gather groups (128 tokens each)

AW0, CW0, W00, W10, W20, IDT0, ONE0 = 0, 512, 1024, 3072, 11264, 13312, 13440
WBIG = 13952
AWF0, CWF0, WSM = 48, 560, 1072

FP32 = mybir.dt.float32
FP32R = mybir.dt.float32r
INT32 = mybir.dt.int32
AF = mybir.ActivationFunctionType
ALU = mybir.AluOpType
AX = mybir.AxisListType

TRACE = False               # set by test.py for profiling
LAST_RESULTS = None

_nc_cache = {}


def _r(ap):
    return ap.bitcast(FP32R)


def _build(nsup, general_sf, rounds=1):
    tok = nsup * SUP
    ng = tok // 128
    nc = bass.Bass("TRN2", target_bir_lowering=False, debug=False,
                   num_devices=NCORE)

    # z is passed feature-major [D, tok] (host-side transpose) so tiles DMA
    # straight into partitions 64..127 with no on-device transpose.
    z_in = nc.dram_tensor("z_in", [D, tok], FP32, kind="ExternalInput").ap()
    idx_in = nc.dram_tensor("idx_in", [128, ng], INT32, kind="ExternalInput").ap()
    emb_in = nc.dram_tensor("emb_in", [V, E], FP32R, kind="ExternalInput").ap()
    wbig_in = nc.dram_tensor("wbig_in", [128, WBIG], FP32R,
                             kind="ExternalInput").ap()
    wsm_in = nc.dram_tensor("wsm_in", [128, WSM], FP32,
                            kind="ExternalInput").ap()
    z_out = nc.dram_tensor("z_out", [D, tok], FP32, kind="ExternalOutput").ap()
    ldj_out = nc.dram_tensor("ldj_out", [1, BPC], FP32, kind="ExternalOutput").ap()

    with tile.TileContext(nc) as tc, ExitStack() as ctx:
        const = ctx.enter_context(tc.tile_pool(name="const", bufs=1))
        zp = ctx.enter_context(tc.tile_pool(name="zp", bufs=2))
        hp = ctx.enter_context(tc.tile_pool(name="hp", bufs=2))
        mlp = ctx.enter_context(tc.tile_pool(name="mlp", bufs=1))
        ch = ctx.enter_context(tc.tile_pool(name="ch", bufs=1))
        io = ctx.enter_context(tc.tile_pool(name="io", bufs=2))
        ps = ctx.enter_context(tc.tile_pool(name="ps", bufs=1, space="PSUM"))
        pm = ctx.enter_context(tc.tile_pool(name="pm", bufs=2, space="PSUM"))
        pe = ctx.enter_context(tc.tile_pool(name="pe", bufs=1, space="PSUM"))

        # ---- constants / weights (resident for the whole kernel) ----
        wbig = const.tile([128, WBIG], FP32R)
        nc.sync.dma_start(wbig[:, :], wbig_in[:, :])
        wsm = const.tile([128, WSM], FP32)
        nc.sync.dma_start(wsm[:, :], wsm_in[:, :])
        aw = wsm[0:64, AWF0:AWF0 + 512]
        cw = wsm[:, CWF0:CWF0 + 512]
        w0 = wbig[:, W00:W00 + 2048]
        w1 = wbig[:, W10:W10 + 8192]
        w2 = wbig[:, W20:W20 + 2048]
        wv = wsm[:, 0:F * 8]
        b1c = wsm[:, F * 8:F * 12]
        idxs = const.tile([128, ng], INT32)
        nc.sync.dma_start(idxs[:, :], idx_in[:, :])
        ident = wbig[:, IDT0:IDT0 + 128]  # identity (f32r), loaded from DRAM
        ones = const.tile([128, 1], FP32)
        nc.vector.memset(ones[:, :], 1.0)
        acc = const.tile([128, nsup * 8], FP32)
        nc.vector.memset(acc[:, :], 0.0)
        acc8 = const.tile([128, BPC], FP32)
        ldj_sb = const.tile([1, BPC], FP32)

        for _rep in range(rounds):
          for st in range(nsup):
            t0 = st * SUP

            # ---- embedding gather + transpose to feature-major ----
            ext_tok = io.tile([128, 512], FP32R, tag="ext")
            for g in range(8):
                nc.gpsimd.indirect_dma_start(
                    out=ext_tok[:, g * 64:(g + 1) * 64],
                    out_offset=None,
                    in_=emb_in[:, :],
                    in_offset=bass.IndirectOffsetOnAxis(
                        ap=idxs[:, st * 8 + g: st * 8 + g + 1], axis=0),
                )
            # transpose via regular matmul: pex = ext_tok^T @ I (avoids
            # transpose-mode matmuls, whose LW slot only fits one sync wait)
            pex = pe.tile([64, 1024], FP32, tag="extps")
            for g in range(8):
                nc.tensor.matmul(pex[0:64, g * 128:(g + 1) * 128],
                                 ext_tok[:, g * 64:(g + 1) * 64],
                                 ident[:, :])

            h0a = hp.tile([128, TT], FP32R, tag="h0a")
            h0b = hp.tile([128, TT], FP32R, tag="h0b")
            nc.vector.tensor_copy(h0a[0:64, :], pex[0:64, 0:512])
            nc.vector.tensor_copy(h0b[0:64, :], pex[0:64, 512:1024])
            extf = hp.tile([128, SUP], FP32, tag="extf")
            nc.vector.tensor_copy(extf[0:64, :], pex[0:64, :])
            nc.sync.dma_start(h0a[96:97, :], wbig_in[0:1, ONE0:ONE0 + 512])
            nc.sync.dma_start(h0b[96:97, :], wbig_in[0:1, ONE0:ONE0 + 512])

            zab = zp.tile([128, SUP], FP32, tag="z")
            nc.sync.dma_start(zab[64:128, :], z_in[:, t0:t0 + 1024])

            for f in range(F):
                cv = f * 8  # wv column base for this flow

                # ======== ExtActNormFlow ========
                pan = ps.tile([128, 1024], FP32, tag="seq")
                nc.tensor.matmul(pan[:, 0:512], aw[:, f * 128:(f + 1) * 128],
                                 extf[0:64, 0:512])
                nc.tensor.matmul(pan[:, 512:1024], aw[:, f * 128:(f + 1) * 128],
                                 extf[0:64, 512:1024])
                # bias part (psum rows 0..63) -> z lanes (rows 64..127)
                bcp = ch.tile([128, 1024], FP32, tag="bcp")
                nc.vector.tensor_copy(bcp[0:64, :], pan[0:64, :])
                bmv = ch.tile([128, 1024], FP32, tag="bmv")
                nc.sync.dma_start(bmv[64:128, :], bcp[0:64, :])
                # scales chain: t1 = tanh(scales + b); es = exp(t1)
                t1 = ch.tile([128, 1024], FP32, tag="t1")
                nc.scalar.activation(t1[64:128, :], pan[64:128, :], AF.Tanh,
                                     bias=wv[64:128, cv:cv + 1],
                                     accum_out=acc[64:128, st * 8 + f * 2:
                                                   st * 8 + f * 2 + 1])
                u = ch.tile([128, 1024], FP32, tag="u")
                nc.scalar.activation(u[64:128, :], t1[64:128, :], AF.Tanh,
                                     scale=0.5)
                d = ch.tile([128, 1024], FP32, tag="d")
                nc.vector.tensor_scalar(d[64:128, :], u[64:128, :], -1.0, 1.0,
                                        ALU.mult, ALU.add)
                rr = ch.tile([128, 1024], FP32, tag="rr")
                nc.vector.reciprocal(rr[64:128, :], d[64:128, :])
                es = ch.tile([128, 1024], FP32, tag="es")
                nc.vector.scalar_tensor_tensor(es[64:128, :], u[64:128, :], 1.0,
                                               rr[64:128, :], ALU.add, ALU.mult)
                zb = ch.tile([128, 1024], FP32, tag="zb")
                nc.vector.scalar_tensor_tensor(zb[64:128, :], bmv[64:128, :],
                                               wv[64:128, cv + 1:cv + 2],
                                               zab[64:128, :], ALU.add, ALU.add)
                nc.vector.tensor_tensor(zab[64:128, :], zb[64:128, :],
                                        es[64:128, :], op=ALU.mult)

                # ======== Invertible 1x1 conv ========
                pc = ps.tile([128, 1024], FP32, tag="seq")
                nc.tensor.matmul(pc[:, 0:512],
                                 cw[64:128, f * 128:(f + 1) * 128],
                                 zab[64:128, 0:512])
                nc.tensor.matmul(pc[:, 512:1024],
                                 cw[64:128, f * 128:(f + 1) * 128],
                                 zab[64:128, 512:1024])
                nc.vector.tensor_copy(zab[64:128, :], pc[64:128, :])

                # ======== Coupling layer MLP ========
                nc.vector.tensor_copy(h0a[64:96, :], zab[64:96, 0:512])
                nc.vector.tensor_copy(h0b[64:96, :], zab[64:96, 512:1024])

                h1 = mlp.tile([128, 4096], FP32R, tag="h1")
                for c in range(4):
                    ph = pm.tile([128, 1024], FP32, tag="mlps")
                    wch = w0[0:97, f * 512 + c * 128: f * 512 + (c + 1) * 128]
                    nc.tensor.matmul(ph[:, 0:512], _r(wch), _r(h0a[0:97, :]))
                    nc.tensor.matmul(ph[:, 512:1024], _r(wch), _r(h0b[0:97, :]))
                    nc.scalar.activation(h1[:, c * 1024:(c + 1) * 1024], ph[:, :],
                                         AF.Gelu)
                h2 = mlp.tile([128, 4096], FP32R, tag="h2")
                for m in range(4):
                    ph = pm.tile([128, 1024], FP32, tag="mlps")
                    for k in range(4):
                        wch = w1[:, f * 2048 + m * 512 + k * 128:
                                 f * 2048 + m * 512 + (k + 1) * 128]
                        nc.tensor.matmul(ph[:, 0:512], _r(wch),
                                         _r(h1[:, k * 1024: k * 1024 + 512]),
                                         start=(k == 0), stop=(k == 3),
                                         skip_group_check=True)
                        nc.tensor.matmul(ph[:, 512:1024], _r(wch),
                                         _r(h1[:, k * 1024 + 512:(k + 1) * 1024]),
                                         start=(k == 0), stop=(k == 3),
                                         skip_group_check=True)
                    nc.scalar.activation(h2[:, m * 1024:(m + 1) * 1024], ph[:, :],
                                         AF.Gelu, bias=b1c[:, f * 4 + m:
                                                           f * 4 + m + 1])
                # L2: psum rows 64..95 = t (chans 32-63), rows 96..127 = s
                pst = ps.tile([128, 1024], FP32, tag="seq")
                for k in range(4):
                    wch = w2[:, f * 512 + k * 128: f * 512 + (k + 1) * 128]
                    nc.tensor.matmul(pst[:, 0:512], _r(wch),
                                     _r(h2[:, k * 1024: k * 1024 + 512]),
                                     start=(k == 0), stop=(k == 3),
                                     skip_group_check=True)
                    nc.tensor.matmul(pst[:, 512:1024], _r(wch),
                                     _r(h2[:, k * 1024 + 512:(k + 1) * 1024]),
                                     start=(k == 0), stop=(k == 3),
                                     skip_group_check=True)

                # ======== coupling update ========
                tcp = ch.tile([128, 1024], FP32, tag="bcp")
                nc.vector.tensor_copy(tcp[64:96, :], pst[64:96, :])
                tmv = ch.tile([128, 1024], FP32, tag="tmv")
                nc.sync.dma_start(tmv[96:128, :], tcp[64:96, :])
                t2 = ch.tile([128, 1024], FP32, tag="t1")
                acol = acc[96:128, st * 8 + f * 2 + 1: st * 8 + f * 2 + 2]
                if general_sf:
                    nc.scalar.activation(t2[96:128, :], pst[96:128, :], AF.Tanh,
                                         bias=wv[96:128, cv + 2:cv + 3],
                                         scale=wv[96:128, cv + 4:cv + 5])
                    s3 = ch.tile([128, 1024], FP32, tag="s3")
                    nc.vector.tensor_scalar(s3[96:128, :], t2[96:128, :],
                                            wv[96:128, cv + 5:cv + 6], None,
                                            ALU.mult, accum_out=acol)
                else:
                    nc.scalar.activation(t2[96:128, :], pst[96:128, :], AF.Tanh,
                                         bias=wv[96:128, cv + 2:cv + 3],
                                         accum_out=acol)
                    s3 = t2
                u2 = ch.tile([128, 1024], FP32, tag="u")
                nc.scalar.activation(u2[96:128, :], s3[96:128, :], AF.Tanh,
                                     scale=0.5)
                d2 = ch.tile([128, 1024], FP32, tag="d")
                nc.vector.tensor_scalar(d2[96:128, :], u2[96:128, :], -1.0, 1.0,
                                        ALU.mult, ALU.add)
                r2 = ch.tile([128, 1024], FP32, tag="rr")
                nc.vector.reciprocal(r2[96:128, :], d2[96:128, :])
                es2 = ch.tile([128, 1024], FP32, tag="es")
                nc.vector.scalar_tensor_tensor(es2[96:128, :], u2[96:128, :], 1.0,
                                               r2[96:128, :], ALU.add, ALU.mult)
                zb2 = ch.tile([128, 1024], FP32, tag="zb")
                nc.vector.scalar_tensor_tensor(zb2[96:128, :], tmv[96:128, :],
                                               wv[96:128, cv + 3:cv + 4],
                                               zab[96:128, :], ALU.add, ALU.add)
                nc.vector.tensor_tensor(zab[96:128, :], zb2[96:128, :],
                                        es2[96:128, :], op=ALU.mult)

            # ---- store z (feature-major; host transposes back) ----
            nc.sync.dma_start(z_out[:, t0:t0 + 1024], zab[64:128, :])

        # ---- fold ldj accumulator to per-batch scalars ----
        npb = nsup * 8 // BPC if nsup * 8 >= BPC else 1
        nc.vector.reduce_sum(acc8[:, :],
                             acc[:, :].rearrange("p (b c) -> p b c", c=npb),
                             axis=AX.X)
        pl = ps.tile([128, 1024], FP32, tag="seq")
        nc.tensor.matmul(pl[0:1, 0:BPC], ones[:, 0:1], acc8[:, :])
        nc.vector.tensor_copy(ldj_sb[0:1, :], pl[0:1, 0:BPC])
        nc.sync.dma_start(ldj_out[:, :], ldj_sb[:, :])

    return nc


def _spread_matmul_waits(nc):
    """Walrus puts a matmul's sync waits on its S3_LW uop, which fits only
    one; move excess waits onto PE engine-NOPs spliced just before it.
    Waits execute in stream order on the PE queue, so semantics match."""
    if getattr(nc, "_waits_spread", False):
        return
    nc._waits_spread = True
    for fn in nc.m.functions:
        for bb in fn.blocks:
            out = []
            changed = False
            for inst in bb.instructions:
                si = inst.sync_info
                if si is not None and len(si.on_wait) > 1:
                    waits = list(si.on_wait)
                    for w in waits[:-1]:
                        nop = mybir.InstEventSemaphore(
                            name=nc.get_next_instruction_name(),
                            engine=inst.engine, ins=[], outs=[])
                        nop.sync_info = type(si)(on_wait=[w], on_update=[])
                        out.append(nop)
                    inst.sync_info = type(si)(on_wait=[waits[-1]],
                                              on_update=list(si.on_update))
                    changed = True
                out.append(inst)
            if changed:
                bb.instructions = out


def _get_nc(nsup, general_sf):
    key = (nsup, general_sf)
    if key not in _nc_cache:
        nc = _build(nsup, general_sf)
        _spread_matmul_waits(nc)
        _nc_cache[key] = nc
    return _nc_cache[key]


def _prep_weights(actnorm_W, actnorm_b, conv_W, scaling_factor,
                  W0, b0, W1, b1, W2, b2):
    """Host-side arrangement of weights into their exact SBUF images."""
    f32 = np.float32
    wbig = np.zeros((128, WBIG), f32)
    wsm = np.zeros((128, WSM), f32)
    aw_h = wbig[0:64, AW0:AW0 + 512]
    cw_h = wbig[:, CW0:CW0 + 512]
    w0_h = wbig[:, W00:W00 + 2048]
    w1_h = wbig[:, W10:W10 + 8192]
    w2_h = wbig[:, W20:W20 + 2048]
    wv_h = wsm[:, 0:F * 8]
    b1_h = wsm[:, F * 8:F * 12]
    awf_h = wsm[0:64, AWF0:AWF0 + 512]
    cwf_h = wsm[:, CWF0:CWF0 + 512]
    aw_h[:, :] = np.concatenate([actnorm_W[f] for f in range(F)], axis=1)
    awf_h[:, :] = aw_h
    wbig[:, IDT0:IDT0 + 128] = np.eye(128, dtype=f32)
    wbig[0, ONE0:ONE0 + 512] = 1.0
    for f in range(F):
        cw_h[64:128, f * 128 + 64:(f + 1) * 128] = conv_W[f]
        cwf_h[64:128, f * 128 + 64:(f + 1) * 128] = conv_W[f]
        w0_aug = np.concatenate([W0[f][64:128], W0[f][0:32], b0[f][None, :]],
                                axis=0)  # [97, 512]
        w0_h[0:97, f * 512:(f + 1) * 512] = w0_aug
        for m in range(4):
            b1_h[:, f * 4 + m] = b1[f][m * 128:(m + 1) * 128]
            for k in range(4):
                w1_h[:, f * 2048 + m * 512 + k * 128:
                     f * 2048 + m * 512 + (k + 1) * 128] = \
                    W1[f][k * 128:(k + 1) * 128, m * 128:(m + 1) * 128]
        w2r = W2[f].reshape(H, 64, 2)
        w2_st = np.concatenate([w2r[:, 32:64, 1], w2r[:, 32:64, 0]],
                               axis=1)  # [512, 64] cols [t | s]
        for k in range(4):
            w2_h[:, f * 512 + k * 128 + 64: f * 512 + (k + 1) * 128] = \
                w2_st[k * 128:(k + 1) * 128, :]
        b2r = b2[f].reshape(64, 2)
        sfa = np.exp(scaling_factor[f].astype(np.float64))[32:64]
        inv = (1.0 / np.maximum(sfa, 1.0)).astype(f32)
        wv_h[64:128, f * 8 + 0] = actnorm_b[f][64:128]
        wv_h[64:128, f * 8 + 1] = actnorm_b[f][0:64]
        wv_h[96:128, f * 8 + 2] = b2r[32:64, 0] * inv
        wv_h[96:128, f * 8 + 3] = b2r[32:64, 1]
        wv_h[96:128, f * 8 + 4] = inv
        wv_h[96:128, f * 8 + 5] = sfa.astype(f32)
    return dict(wbig_in=wbig, wsm_in=wsm)


def kernel(z, categ, embed, actnorm_W, actnorm_b, conv_W, scaling_factor,
           W0, b0, W1, b1, W2, b2):
    global LAST_RESULTS
    z = np.ascontiguousarray(np.asarray(z, np.float32))
    categ = np.ascontiguousarray(np.asarray(categ).astype(np.int32))
    embed = np.ascontiguousarray(np.asarray(embed, np.float32))
    args = [np.asarray(a, np.float32) for a in
            (actnorm_W, actnorm_b, conv_W, scaling_factor, W0, b0, W1, b1,
             W2, b2)]
    actnorm_W, actnorm_b, conv_W, scaling_factor, W0, b0, W1, b1, W2, b2 = args

    general_sf = bool(np.any(scaling_factor != 0.0))
    nc = _get_nc(NSUP, general_sf)
    wdict = _prep_weights(actnorm_W, actnorm_b, conv_W, scaling_factor,
                          W0, b0, W1, b1, W2, b2)

    in_maps = []
    for c in range(NCORE):
        zc = z[c * BPC:(c + 1) * BPC].reshape(TOK, D)
        cc = categ[c * BPC:(c + 1) * BPC].reshape(TOK)
        idx_h = np.ascontiguousarray(cc.reshape(NG, 128).T)
        m = dict(z_in=np.ascontiguousarray(zc.T), idx_in=idx_h, emb_in=embed)
        m.update(wdict)
        in_maps.append(m)

    res = run_bass_kernel_spmd(nc, in_maps, core_ids=list(range(NCORE)),
                               trace=TRACE)
    LAST_RESULTS = res

    z_full = np.concatenate(
        [np.ascontiguousarray(res.results[c]["z_out"].T).reshape(BPC, S, D)
         for c in range(NCORE)],
        axis=0)
    ldj_dev = np.concatenate(
        [res.results[c]["ldj_out"].reshape(BPC) for c in range(NCORE)])
    ldj_const = S * sum(np.linalg.slogdet(conv_W[f].astype(np.float64))[1]
                        for f in range(F))
    ldj = (ldj_dev.astype(np.float64) + ldj_const).astype(np.float32)
    return z_full, ldj
